# revision 25
# baseline (speedup 1.0000x reference)
"""Trainium2 Bass kernel for Brain3DQTUNNetwork (gnn_message_passing).

Per core: y-slab of 8 planes, 128 partitions p = h*64 + x (h = y-half).
Weights stored SOURCE-x-aligned per offset slot, so the SpMV reads the prev
field with only (dy,dz) free-dim shifts; per-slot products are realigned to
destinations and k-reduced by 24 tiny PE matmuls (banded 0/1 shift matrices
/ identity) accumulating in PSUM.  Products/weights run fp16 (DVE 2x_1p /
4x_2p); V/neuron path fp32.  Weight decay is folded into a global scale
c_t = (1-wd)^t so the STDP update is a plain fp16 add + 2-op clip.
Cross-core traffic: one AllGather of 2-row boundary strips per step,
overlapped with halo-free STDP and the NEXT step's halo-free syn products
(software pipelining).
"""

import os
import sys

sys.path.insert(0, "/opt/trn_rl_repo")

import numpy as np

import concourse.bass as bass
import concourse.bacc as bacc
import concourse.mybir as mybir
import concourse.tile as tile
from concourse import bass_utils
from bass_rust import AP as RawAP

# ---- problem constants (hardcoded; kernel.py must be self-contained) ----
GRID = (64, 64, 64)
NX, NY, NZ = GRID
N = NX * NY * NZ
RADIUS = 2
NCORES = 8
YS = NY // NCORES  # 8 y-planes per core
YH = YS // 2       # 4 rows per partition half

TAU = 20.0
REST_V = -65.0
EXC_THR = -50.0
INH_THR = -70.0
RESET_V = -65.0
ETA_LTP, ETA_LTD, WDECAY = 0.01, 0.005, 1e-05

DECAY = float(np.exp(np.float32(-1.0 / np.float32(TAU))).astype(np.float32))
ONE_MINUS_DECAY = float(np.float32(1.0) - np.float32(DECAY))
MIDPOINT = (EXC_THR + INH_THR) / 2.0  # -60.0

# slot table: (dx, dy, dz).  Halo-free (dy==0) ranges: [0:3),[5:8),[10:14),
# [22:24); each (dx,dy,dz-run) group is k-consecutive.
SLOTS = [
    (1, 0, 1), (1, 0, 0), (1, 0, -1),          # k0-2
    (1, 1, 0), (1, -1, 0),                     # k3, k4
    (-1, 0, 1), (-1, 0, 0), (-1, 0, -1),       # k5-7
    (-1, 1, 0), (-1, -1, 0),                   # k8, k9
    (0, 0, 2), (0, 0, 1),                      # k10-11
    (0, 0, -1), (0, 0, -2),                    # k12-13
    (0, 1, 1), (0, 1, 0), (0, 1, -1),          # k14-16
    (0, -1, 1), (0, -1, 0), (0, -1, -1),       # k17-19
    (0, 2, 0), (0, -2, 0),                     # k20, k21
    (2, 0, 0), (-2, 0, 0),                     # k22, k23
]
NOFF = len(SLOTS)  # 24

# mult groups.  run: (k0, L, dy, dz_start) with dz descending (field
# k-stride +1).  pair: (ka, kb, dy, dz0, fstride, prev) — two slots sharing
# one instruction, field k-stride `fstride` (0 = broadcast), prev side
# `prev` in {0 (FPREV), 'p01', 'p23'} (PVX slice pairs).
GROUPS_HF = [
    ("run", 0, 3, 0, 1, 1), ("run", 5, 3, 0, 1, -1),
    ("run", 10, 2, 0, 2, 0), ("run", 12, 2, 0, -1, 0),
    ("pair", 22, 23, 0, 0, 0, "p23"),
]
GROUPS_H = [
    ("run", 14, 3, 1, 1, 0), ("run", 17, 3, -1, 1, 0),
    ("pair", 3, 8, 1, 0, 0, "p01"),
    ("pair", 4, 9, -1, 0, 0, "p01"),
    ("pair", 20, 21, 2, 0, 4 * 68, 0),  # dy=+2 base; +272 elems = dy=-2 rows
]
PVX_SLICE = {1: 0, -1: 1, 2: 2, -2: 3}
SM_SLICE = {1: 0, -1: 1, 2: 2, -2: 3, 0: 4}

# PE reduction order: halo-free slots first, grouped by shift matrix.
MM_ORDER_HF = [10, 11, 12, 13, 0, 1, 2, 5, 6, 7, 22, 23]
MM_ORDER_H = [14, 15, 16, 17, 18, 19, 3, 4, 8, 9, 20, 21]

FZ = NZ + 4        # 68 field z cols
FR = 2 * YH        # 8 field rows
FFREE = FR * FZ    # 544
CH = YH * NZ       # 256 own cells per partition

F32 = mybir.dt.float32
U8 = mybir.dt.uint8

USE_F16 = bool(int(os.environ.get("BRAIN_F16", "1")))
DT = mybir.dt.float16 if USE_F16 else mybir.dt.float32
NPDT = np.float16 if USE_F16 else np.float32

_CACHE = {}


def _overlap_ap(view, kstride, ksize):
    """Insert a k dim (kstride in free elems) after the partition dim."""
    ap = [list(d) for d in view.ap]
    ap.insert(1, [kstride, ksize])
    return RawAP(tensor=view.tensor, offset=view.offset, ap=ap)


def _build_graph(nsteps):
    nc = bacc.Bacc(
        "TRN2",
        target_bir_lowering=False,
        debug=False,
        enable_asserts=True,
        num_devices=NCORES,
    )
    P128 = 2 * NX
    w0_d = nc.dram_tensor("w0", [P128, NOFF * CH], DT, kind="ExternalInput").ap()
    xin_d = nc.dram_tensor("xin", [P128, nsteps * CH], F32, kind="ExternalInput").ap()
    msk_d = nc.dram_tensor("msk", [P128, 2], F32, kind="ExternalInput").ap()
    sm_d = nc.dram_tensor("sm", [P128, 5 * P128], DT, kind="ExternalInput").ap()
    spk_d = nc.dram_tensor("spk", [nsteps, P128, CH], F32, kind="ExternalOutput").ap()

    AT = mybir.ActivationFunctionType
    ALU = mybir.AluOpType

    c = [float(np.float64(1.0 - WDECAY) ** t) for t in range(nsteps)]

    with tile.TileContext(nc) as tc, tc.tile_pool(
        name="state", bufs=1
    ) as st, tc.tile_pool(name="psum", bufs=1, space="PSUM") as ps, tc.tile_pool(
        name="dram", bufs=1, space="DRAM"
    ) as dr:
        W = st.tile([P128, NOFF * CH], DT, name="W")
        P = st.tile([P128, NOFF * CH], DT, name="P")
        FA = st.tile([P128, FFREE], DT, name="FA")
        FB = st.tile([P128, FFREE], DT, name="FB")
        QF = st.tile([P128, FFREE], DT, name="QF")
        PVX2 = [st.tile([P128, 4 * CH], DT, name=f"PVX{i}") for i in range(2)]
        SM = st.tile([P128, 5 * P128], DT, name="SM")
        XINP = st.tile([P128, nsteps * CH], F32, name="XINP")
        V = st.tile([P128, CH], F32, name="V")
        SYN = st.tile([P128, CH], F32, name="SYN")
        SS = [st.tile([P128, CH], F32, name=f"S{i}") for i in range(2)]
        SI = st.tile([P128, CH], U8, name="SI")
        II = st.tile([P128, CH], F32, name="II")
        G = st.tile([P128, CH], F32, name="G")
        E = st.tile([P128, CH], F32, name="E")
        RST = st.tile([P128, CH], F32, name="RST")
        B30 = st.tile([P128, 1], F32, name="B30")
        MSKB = st.tile([P128, 2], F32, name="MSKB")
        SA = st.tile([P128, 2], F32, name="SA")
        TDP = ps.tile([P128, CH], F32, name="TDP")

        def f3(t):
            return t.rearrange("p (r z) -> p r z", z=FZ)

        def c3(t):
            return t.rearrange("p (y z) -> p y z", z=NZ)

        def w4(t, k0, L):
            return t.rearrange("p (k y z) -> p k y z", k=NOFF, z=NZ)[:, k0 : k0 + L]

        def wf(t, k0, k1):
            return t.rearrange("p (k f) -> p k f", k=NOFF)[:, k0:k1]

        def blockpair(t, r0, r1, pair0):
            v = wf(t, r0, r1)
            ap = [list(d) for d in v.ap]
            ap.insert(1, [(pair0 - r0) * CH, 2])
            return RawAP(tensor=v.tensor, offset=v.offset, ap=ap)

        def fint(t):
            return f3(t)[:, 2 : 2 + YH, 2 : 2 + NZ]

        def grp_field(t, L, dy, dz0, kstride=1):
            base = f3(t)[:, 2 - dy : 2 - dy + YH, 2 - dz0 : 2 - dz0 + NZ]
            if L == 1:
                return base.unsqueeze(1)
            return _overlap_ap(base, kstride, L)

        def pairk(t, ka, kb):
            v = w4(t, ka, 1)
            ap = [list(d) for d in v.ap]
            ap[1] = [(kb - ka) * CH, 2]
            return RawAP(tensor=v.tensor, offset=v.offset, ap=ap)

        def prev_bcast(pvx, t, L, dx):
            if dx == 0:
                v = fint(t)
            else:
                v = c3(pvx.rearrange("p (s f) -> p s f", s=4)[:, PVX_SLICE[dx]])
            return v.unsqueeze(1).to_broadcast([P128, L, YH, NZ])

        def pvx4(pvx):
            return pvx.rearrange("p (s y z) -> p s y z", s=4, z=NZ)

        def xshift_dma(eng, dst, src, dx):
            for h in (0, 1):
                a, b = h * NX + max(0, dx), h * NX + NX + min(0, dx)
                eng.dma_start(dst[a:b], src[a - dx : b - dx])

        smv = SM.rearrange("p (s m) -> p s m", s=5)
        Pk = P.rearrange("p (k f) -> p k f", k=NOFF)

        def syn_mults(FPREV, groups):
            for g in groups:
                if g[0] == "run":
                    _, k0, L, dy, dz0, _ = g
                    nc.vector.tensor_tensor(
                        w4(P, k0, L), w4(W, k0, L), grp_field(FPREV, L, dy, dz0),
                        ALU.mult,
                    )
                else:
                    _, ka, kb, dy, dz0, fs, _ = g
                    nc.vector.tensor_tensor(
                        pairk(P, ka, kb), pairk(W, ka, kb),
                        grp_field(FPREV, 2, dy, dz0, kstride=fs),
                        ALU.mult,
                    )

        def stdp_mults(FPREV, pvx, groups):
            for g in groups:
                if g[0] == "run":
                    _, k0, L, dy, dz0, pdx = g
                    nc.vector.tensor_tensor(
                        w4(P, k0, L), prev_bcast(pvx, FPREV, L, pdx),
                        grp_field(QF, L, dy, dz0), ALU.mult,
                    )
                else:
                    _, ka, kb, dy, dz0, fs, prev = g
                    if prev == "p01":
                        pv = pvx4(pvx)[:, 0:2]
                    elif prev == "p23":
                        pv = pvx4(pvx)[:, 2:4]
                    else:
                        pv = fint(FPREV).unsqueeze(1).to_broadcast(
                            [P128, 2, YH, NZ]
                        )
                    nc.vector.tensor_tensor(
                        pairk(P, ka, kb), pv,
                        grp_field(QF, 2, dy, dz0, kstride=fs),
                        ALU.mult,
                    )

        def w_update(ranges, hi_t):
            for (r0, r1), pair in ranges:
                if pair is not None:
                    wv = blockpair(W, r0, r1, pair)
                    pv = blockpair(P, r0, r1, pair)
                else:
                    wv, pv = wf(W, r0, r1), wf(P, r0, r1)
                nc.vector.tensor_tensor(wv, wv, pv, ALU.add)
                nc.vector.tensor_scalar(wv, wv, hi_t, 0.0, ALU.min, ALU.max)

        RANGES_HF = (((0, 3), 5), ((10, 14), None), ((22, 24), None))

        # ---- init ----
        nc.vector.memset(FA[:], 0.0)
        nc.vector.memset(FB[:], 0.0)
        nc.vector.memset(PVX2[0][:], 0.0)
        nc.vector.memset(PVX2[1][:], 0.0)
        nc.vector.memset(V[:], REST_V)
        nc.vector.memset(RST[:], RESET_V)
        nc.vector.memset(B30[:], -0.5 * MIDPOINT)
        nc.sync.dma_start(W[:], w0_d[:])
        nc.sync.dma_start(SM[:], sm_d[:])
        nc.sync.dma_start(MSKB[:], msk_d[:])
        XV = XINP.rearrange("p (t f) -> p t f", t=nsteps)
        xin_v = xin_d.rearrange("p (t f) -> p t f", t=nsteps)
        nc.sync.dma_start(XV[:, 0:1], xin_v[:, 0:1])
        nc.sync.dma_start(XV[:, 1:], xin_v[:, 1:])

        pid = nc.sync.partition_id()
        offL = nc.sync.snap((pid + NCORES - 1) % NCORES, min_val=0, max_val=NCORES - 1)
        offR = nc.sync.snap((pid + 1) % NCORES, min_val=0, max_val=NCORES - 1)

        fields = [FA, FB]
        for t in range(nsteps):
            FPREV = fields[t % 2]
            FOUT = fields[(t + 1) % 2]
            last = t == nsteps - 1
            S = SS[t % 2]
            pvx = PVX2[t % 2]       # this step's stdp reads pvx
            pvx_next = PVX2[(t + 1) % 2]

            # ---- syn: halo-dependent products (HF ones were emitted in the
            # previous iteration's overlap window), PE k-reduction ----
            if t > 0:
                syn_mults(FPREV, GROUPS_H)
                order = MM_ORDER_HF + MM_ORDER_H
                for i, k in enumerate(order):
                    nc.tensor.matmul(
                        TDP[:], smv[:, SM_SLICE[SLOTS[k][0]]], Pk[:, k],
                        start=(i == 0), stop=(i == len(order) - 1),
                    )
                s_t = float(np.float32(ONE_MINUS_DECAY * c[t - 1]))
                nc.vector.scalar_tensor_tensor(
                    SYN[:], TDP[:], s_t, XV[:, t], ALU.mult, ALU.add
                )
                nc.vector.scalar_tensor_tensor(
                    V[:], V[:], DECAY, SYN[:], ALU.mult, ALU.add
                )
            else:
                nc.vector.scalar_tensor_tensor(
                    V[:], V[:], DECAY, XV[:, t], ALU.mult, ALU.add
                )

            # ---- neuron update ----
            nc.vector.tensor_scalar(S[:], V[:], EXC_THR, None, ALU.is_ge)
            nc.sync.dma_start(spk_d[t], S[:])
            if last:
                continue
            nc.vector.tensor_scalar(SI[:], V[:], EXC_THR, None, ALU.is_ge)
            nc.vector.tensor_scalar(II[:], V[:], INH_THR, None, ALU.is_le)
            nc.scalar.activation(G[:], V[:], AT.Sigmoid, bias=B30[:, 0:1], scale=0.5)
            nc.vector.tensor_tensor(E[:], S[:], II[:], ALU.subtract)
            nc.vector.tensor_tensor(E[:], G[:], E[:], ALU.add)
            # out = clip01(...): boundary-strip rows first so the AllGather
            # input DMAs launch before the interior is written
            nc.vector.tensor_scalar(
                fint(FOUT)[0:NX, 0:2], c3(E)[0:NX, 0:2], 1.0, 0.0, ALU.min, ALU.max
            )
            nc.vector.tensor_scalar(
                fint(FOUT)[NX:P128, 2:4], c3(E)[NX:P128, 2:4],
                1.0, 0.0, ALU.min, ALU.max,
            )
            # ---- boundary strips -> AllGather (unmasked; edge-core wrap
            # garbage is neutralized by W'=0 on syn and the masked QF-halo
            # scale on STDP) ----
            agin = dr.tile([P128, 2 * NZ], DT, name=f"agin{t}")
            agout = dr.tile(
                [NCORES * P128, 2 * NZ], DT, addr_space="Shared", name=f"agout{t}"
            )
            agv = agin.rearrange("p (r z) -> p r z", z=NZ)
            nc.sync.dma_start(agv[0:NX], f3(FOUT)[0:NX, 2:4, 2 : 2 + NZ])
            nc.sync.dma_start(agv[NX:P128], f3(FOUT)[NX:P128, 4:6, 2 : 2 + NZ])
            nc.vector.tensor_scalar(
                fint(FOUT)[0:NX, 2:4], c3(E)[0:NX, 2:4], 1.0, 0.0, ALU.min, ALU.max
            )
            nc.vector.tensor_scalar(
                fint(FOUT)[NX:P128, 0:2], c3(E)[NX:P128, 0:2],
                1.0, 0.0, ALU.min, ALU.max,
            )
            nc.vector.copy_predicated(V[:], SI[:], RST[:])
            nc.gpsimd.collective_compute(
                "AllGather",
                ALU.bypass,
                replica_groups=[list(range(NCORES))],
                ins=[agin.opt()],
                outs=[agout.opt()],
            )
            nc.sync.dma_start(f3(FOUT)[0:NX, 6:8, 2 : 2 + NZ],
                              f3(FOUT)[NX:P128, 2:4, 2 : 2 + NZ])
            nc.sync.dma_start(f3(FOUT)[NX:P128, 0:2, 2 : 2 + NZ],
                              f3(FOUT)[0:NX, 4:6, 2 : 2 + NZ])

            # ---- overlap window: halo-free STDP + next-step prefetches ----
            do_stdp = t > 0
            if do_stdp:
                a_t = float(np.float32((ETA_LTP + ETA_LTD) / c[t]))
                b_t = float(np.float32(-ETA_LTD / c[t]))
                hi_t = float(np.float32(1.0 / c[t]))
                qf3 = f3(QF)
                fo3 = f3(FOUT)
                nc.vector.tensor_scalar(
                    qf3[:, 2:6], fo3[:, 2:6], a_t, b_t, ALU.mult, ALU.add
                )
                # per-step masked QF-halo scales (zero invalid halo sources)
                nc.vector.tensor_scalar(SA[:], MSKB[:], a_t, None, ALU.mult)
                stdp_mults(FPREV, pvx, GROUPS_HF)
                w_update(RANGES_HF, hi_t)

            # prefetch x-shifted prev chunks for next step's stdp
            if t + 1 < nsteps - 1:
                pvs = pvx_next.rearrange("p (s f) -> p s f", s=4)
                for dxv, sl in PVX_SLICE.items():
                    xshift_dma(nc.scalar, pvs[:, sl], fint(FOUT), -dxv)

            # next step's halo-free syn products (fills the collective wait)
            if t + 1 < nsteps:
                syn_mults(FOUT, GROUPS_HF)

            # ---- halo in from neighbors ----
            agf = agout.rearrange("p (r z) -> p r z", z=NZ)
            nc.sync.dma_start(
                f3(FOUT)[0:NX, 0:2, 2 : 2 + NZ],
                agf[bass.ds(offL * P128 + NX, NX)],
            )
            nc.sync.dma_start(
                f3(FOUT)[NX:P128, 6:8, 2 : 2 + NZ],
                agf[bass.ds(offR * P128, NX)],
            )

            # ---- halo-dependent STDP (interleaved mult/update for
            # per-range pipelining behind the two halo-in DMAs) ----
            if do_stdp:
                nc.vector.tensor_scalar(
                    qf3[:, 0:2], fo3[:, 0:2], SA[:, 0:1], b_t, ALU.mult, ALU.add
                )
                nc.vector.tensor_scalar(
                    qf3[:, 6:8], fo3[:, 6:8], SA[:, 1:2], b_t, ALU.mult, ALU.add
                )
                stdp_mults(FPREV, pvx, [("run", 14, 3, 1, 1, 0)])
                w_update((((14, 17), None),), hi_t)
                stdp_mults(FPREV, pvx, [("run", 17, 3, -1, 1, 0)])
                w_update((((17, 20), None),), hi_t)
                stdp_mults(FPREV, pvx,
                           [("pair", 3, 8, 1, 0, 0, "p01"),
                            ("pair", 4, 9, -1, 0, 0, "p01")])
                w_update((((3, 5), 8),), hi_t)
                stdp_mults(FPREV, pvx, [("pair", 20, 21, 2, 0, 4 * 68, 0)])
                w_update((((20, 22), None),), hi_t)

    nc.compile()
    return nc


def _shard_inputs(external_input, edge_values, edge_rows, edge_cols, nsteps):
    ext = np.ascontiguousarray(np.asarray(external_input, dtype=np.float32))[:nsteps]
    vals = np.asarray(edge_values, dtype=np.float32)
    rows = np.asarray(edge_rows, dtype=np.int64)
    cols = np.asarray(edge_cols, dtype=np.int64)

    dlin = cols - rows
    offs_lin = np.array([d[0] * NY * NZ + d[1] * NZ + d[2] for d in SLOTS])
    k_of = {int(v): i for i, v in enumerate(offs_lin)}
    ke = np.array([k_of[int(v)] for v in dlin], dtype=np.int64)
    Wd = np.zeros((NOFF, N), dtype=np.float32)
    Wd[ke, cols] = vals
    Wd = Wd.reshape(NOFF, NX, NY, NZ)

    Wsrc = np.zeros_like(Wd)
    for k, (dx, _, _) in enumerate(SLOTS):
        if dx >= 0:
            Wsrc[k, : NX - dx] = Wd[k, dx:]
        else:
            Wsrc[k, -dx:] = Wd[k, : NX + dx]

    ext4 = ext.reshape(nsteps, NX, NY, NZ) * np.float32(ONE_MINUS_DECAY)

    # shift matrices: slices (+1,-1,+2,-2,identity); SM_s[p,m]=1 iff m=p+dx
    sm = np.zeros((2 * NX, 5, 2 * NX), dtype=np.float32)
    for s, dxv in enumerate((1, -1, 2, -2, 0)):
        for h in (0, 1):
            for xs in range(NX):
                xm = xs + dxv
                if 0 <= xm < NX:
                    sm[h * NX + xs, s, h * NX + xm] = 1.0
    sm = sm.reshape(2 * NX, 5 * 2 * NX).astype(NPDT)

    in_maps = []
    for cidx in range(NCORES):
        ylo = cidx * YS
        sub = Wsrc[:, :, ylo : ylo + YS, :]
        tr = sub.transpose(1, 0, 2, 3)
        wc = np.concatenate(
            [tr[:, :, :YH, :].reshape(NX, NOFF * CH),
             tr[:, :, YH:, :].reshape(NX, NOFF * CH)], axis=0
        ).astype(NPDT)
        esub = ext4[:, :, ylo : ylo + YS, :].transpose(1, 0, 2, 3)
        xc = np.concatenate(
            [esub[:, :, :YH, :].reshape(NX, nsteps * CH),
             esub[:, :, YH:, :].reshape(NX, nsteps * CH)], axis=0
        ).astype(np.float32)
        # halo-validity masks: col 0 gates field rows 0:2 (h0 <- left
        # neighbor, h1 <- intra); col 1 gates rows 6:8 (h0 <- intra,
        # h1 <- right neighbor)
        msk = np.ones((2 * NX, 2), dtype=np.float32)
        if cidx == 0:
            msk[:NX, 0] = 0.0
        if cidx == NCORES - 1:
            msk[NX:, 1] = 0.0
        in_maps.append(
            {"w0": np.ascontiguousarray(wc), "xin": np.ascontiguousarray(xc),
             "msk": msk, "sm": sm}
        )
    return in_maps


def kernel(external_input, edge_values, edge_rows, edge_cols, num_steps):
    nsteps = int(num_steps)
    if nsteps not in _CACHE:
        _CACHE[nsteps] = _build_graph(nsteps)
    nc = _CACHE[nsteps]

    in_maps = _shard_inputs(external_input, edge_values, edge_rows, edge_cols, nsteps)
    res = bass_utils.run_bass_kernel_spmd(
        nc,
        in_maps,
        core_ids=list(range(NCORES)),
        trace=bool(int(os.environ.get("BRAIN_TRACE", "0"))),
    )

    out = np.empty((nsteps, NX, NY, NZ), dtype=np.float32)
    for cidx in range(NCORES):
        ylo = cidx * YS
        spk = res.results[cidx]["spk"].reshape(nsteps, 2, NX, YH, NZ)
        out[:, :, ylo : ylo + YH, :] = spk[:, 0]
        out[:, :, ylo + YH : ylo + YS, :] = spk[:, 1]
    kernel.last_results = res
    return out.reshape(nsteps, N)


# revision 27
# speedup vs baseline: 1.0100x; 1.0100x over previous
"""Trainium2 Bass kernel for Brain3DQTUNNetwork (gnn_message_passing).

Per core: y-slab of 8 planes, 128 partitions p = h*64 + x (h = y-half).
Weights stored SOURCE-x-aligned per offset slot, so the SpMV reads the prev
field with only (dy,dz) free-dim shifts; per-slot products are realigned to
destinations and k-reduced by 24 tiny PE matmuls (banded 0/1 shift matrices
/ identity) accumulating in PSUM.  Products/weights run fp16 (DVE 2x_1p /
4x_2p); V/neuron path fp32.  Weight decay is folded into a global scale
c_t = (1-wd)^t so the STDP update is a plain fp16 add + 2-op clip.
Cross-core traffic: one AllGather of 2-row boundary strips per step,
overlapped with halo-free STDP and the NEXT step's halo-free syn products
(software pipelining).
"""

import os
import sys

sys.path.insert(0, "/opt/trn_rl_repo")

import numpy as np

import concourse.bass as bass
import concourse.bacc as bacc
import concourse.mybir as mybir
import concourse.tile as tile
from concourse import bass_utils
from bass_rust import AP as RawAP

# ---- problem constants (hardcoded; kernel.py must be self-contained) ----
GRID = (64, 64, 64)
NX, NY, NZ = GRID
N = NX * NY * NZ
RADIUS = 2
NCORES = 8
YS = NY // NCORES  # 8 y-planes per core
YH = YS // 2       # 4 rows per partition half

TAU = 20.0
REST_V = -65.0
EXC_THR = -50.0
INH_THR = -70.0
RESET_V = -65.0
ETA_LTP, ETA_LTD, WDECAY = 0.01, 0.005, 1e-05

DECAY = float(np.exp(np.float32(-1.0 / np.float32(TAU))).astype(np.float32))
ONE_MINUS_DECAY = float(np.float32(1.0) - np.float32(DECAY))
MIDPOINT = (EXC_THR + INH_THR) / 2.0  # -60.0

# slot table: (dx, dy, dz).  Halo-free (dy==0) ranges: [0:3),[5:8),[10:14),
# [22:24); each (dx,dy,dz-run) group is k-consecutive.
SLOTS = [
    (1, 0, 1), (1, 0, 0), (1, 0, -1),          # k0-2
    (1, 1, 0), (1, -1, 0),                     # k3, k4
    (-1, 0, 1), (-1, 0, 0), (-1, 0, -1),       # k5-7
    (-1, 1, 0), (-1, -1, 0),                   # k8, k9
    (0, 0, 2), (0, 0, 1),                      # k10-11
    (0, 0, -1), (0, 0, -2),                    # k12-13
    (0, 1, 1), (0, 1, 0), (0, 1, -1),          # k14-16
    (0, -1, 1), (0, -1, 0), (0, -1, -1),       # k17-19
    (0, 2, 0), (0, -2, 0),                     # k20, k21
    (2, 0, 0), (-2, 0, 0),                     # k22, k23
]
NOFF = len(SLOTS)  # 24

# mult groups.  run: (k0, L, dy, dz_start) with dz descending (field
# k-stride +1).  pair: (ka, kb, dy, dz0, fstride, prev) — two slots sharing
# one instruction, field k-stride `fstride` (0 = broadcast), prev side
# `prev` in {0 (FPREV), 'p01', 'p23'} (PVX slice pairs).
GROUPS_HF = [
    ("run", 0, 3, 0, 1, 1), ("run", 5, 3, 0, 1, -1),
    ("run", 10, 2, 0, 2, 0), ("run", 12, 2, 0, -1, 0),
    ("pair", 22, 23, 0, 0, 0, "p23"),
]
GROUPS_H = [
    ("run", 14, 3, 1, 1, 0), ("run", 17, 3, -1, 1, 0),
    ("pair", 3, 8, 1, 0, 0, "p01"),
    ("pair", 4, 9, -1, 0, 0, "p01"),
    ("pair", 20, 21, 2, 0, 4 * 68, 0),  # dy=+2 base; +272 elems = dy=-2 rows
]
PVX_SLICE = {1: 0, -1: 1, 2: 2, -2: 3}
SM_SLICE = {1: 0, -1: 1, 2: 2, -2: 3, 0: 4}

# PE reduction order: halo-free slots first, grouped by shift matrix.
MM_ORDER_HF = [10, 11, 12, 13, 0, 1, 2, 5, 6, 7, 22, 23]
MM_ORDER_H = [14, 15, 16, 17, 18, 19, 3, 4, 8, 9, 20, 21]

FZ = NZ + 4        # 68 field z cols
FR = 2 * YH        # 8 field rows
FFREE = FR * FZ    # 544
CH = YH * NZ       # 256 own cells per partition

F32 = mybir.dt.float32
U8 = mybir.dt.uint8

USE_F16 = bool(int(os.environ.get("BRAIN_F16", "1")))
DT = mybir.dt.float16 if USE_F16 else mybir.dt.float32
NPDT = np.float16 if USE_F16 else np.float32

_CACHE = {}


def _overlap_ap(view, kstride, ksize):
    """Insert a k dim (kstride in free elems) after the partition dim."""
    ap = [list(d) for d in view.ap]
    ap.insert(1, [kstride, ksize])
    return RawAP(tensor=view.tensor, offset=view.offset, ap=ap)


def _build_graph(nsteps):
    nc = bacc.Bacc(
        "TRN2",
        target_bir_lowering=False,
        debug=False,
        enable_asserts=True,
        num_devices=NCORES,
    )
    P128 = 2 * NX
    w0_d = nc.dram_tensor("w0", [P128, NOFF * CH], DT, kind="ExternalInput").ap()
    xin_d = nc.dram_tensor("xin", [P128, nsteps * CH], F32, kind="ExternalInput").ap()
    msk_d = nc.dram_tensor("msk", [P128, 2], F32, kind="ExternalInput").ap()
    sm_d = nc.dram_tensor("sm", [P128, 5 * P128], DT, kind="ExternalInput").ap()
    spk_d = nc.dram_tensor("spk", [nsteps, P128, CH], F32, kind="ExternalOutput").ap()

    AT = mybir.ActivationFunctionType
    ALU = mybir.AluOpType

    c = [float(np.float64(1.0 - WDECAY) ** t) for t in range(nsteps)]

    with tile.TileContext(nc) as tc, tc.tile_pool(
        name="state", bufs=1
    ) as st, tc.tile_pool(name="psum", bufs=1, space="PSUM") as ps, tc.tile_pool(
        name="dram", bufs=1, space="DRAM"
    ) as dr:
        W = st.tile([P128, NOFF * CH], DT, name="W")
        P = st.tile([P128, NOFF * CH], DT, name="P")
        FA = st.tile([P128, FFREE], DT, name="FA")
        FB = st.tile([P128, FFREE], DT, name="FB")
        QF = st.tile([P128, FFREE], DT, name="QF")
        PVX2 = [st.tile([P128, 4 * CH], DT, name=f"PVX{i}") for i in range(2)]
        SM = st.tile([P128, 5 * P128], DT, name="SM")
        XINP = st.tile([P128, nsteps * CH], F32, name="XINP")
        V = st.tile([P128, CH], F32, name="V")
        SYN = st.tile([P128, CH], F32, name="SYN")
        SS = [st.tile([P128, CH], F32, name=f"S{i}") for i in range(2)]
        SI = st.tile([P128, CH], U8, name="SI")
        II = st.tile([P128, CH], F32, name="II")
        G = st.tile([P128, CH], F32, name="G")
        E = st.tile([P128, CH], F32, name="E")
        RST = st.tile([P128, CH], F32, name="RST")
        B30 = st.tile([P128, 1], F32, name="B30")
        MSKB = st.tile([P128, 2], F32, name="MSKB")
        SA = st.tile([P128, 2], F32, name="SA")
        TDP = ps.tile([P128, CH], F32, name="TDP")

        def f3(t):
            return t.rearrange("p (r z) -> p r z", z=FZ)

        def c3(t):
            return t.rearrange("p (y z) -> p y z", z=NZ)

        def w4(t, k0, L):
            return t.rearrange("p (k y z) -> p k y z", k=NOFF, z=NZ)[:, k0 : k0 + L]

        def wf(t, k0, k1):
            return t.rearrange("p (k f) -> p k f", k=NOFF)[:, k0:k1]

        def blockpair(t, r0, r1, pair0):
            v = wf(t, r0, r1)
            ap = [list(d) for d in v.ap]
            ap.insert(1, [(pair0 - r0) * CH, 2])
            return RawAP(tensor=v.tensor, offset=v.offset, ap=ap)

        def fint(t):
            return f3(t)[:, 2 : 2 + YH, 2 : 2 + NZ]

        def grp_field(t, L, dy, dz0, kstride=1):
            base = f3(t)[:, 2 - dy : 2 - dy + YH, 2 - dz0 : 2 - dz0 + NZ]
            if L == 1:
                return base.unsqueeze(1)
            return _overlap_ap(base, kstride, L)

        def pairk(t, ka, kb):
            v = w4(t, ka, 1)
            ap = [list(d) for d in v.ap]
            ap[1] = [(kb - ka) * CH, 2]
            return RawAP(tensor=v.tensor, offset=v.offset, ap=ap)

        def prev_bcast(pvx, t, L, dx):
            if dx == 0:
                v = fint(t)
            else:
                v = c3(pvx.rearrange("p (s f) -> p s f", s=4)[:, PVX_SLICE[dx]])
            return v.unsqueeze(1).to_broadcast([P128, L, YH, NZ])

        def pvx4(pvx):
            return pvx.rearrange("p (s y z) -> p s y z", s=4, z=NZ)

        def xshift_dma(eng, dst, src, dx):
            for h in (0, 1):
                a, b = h * NX + max(0, dx), h * NX + NX + min(0, dx)
                eng.dma_start(dst[a:b], src[a - dx : b - dx])

        smv = SM.rearrange("p (s m) -> p s m", s=5)
        Pk = P.rearrange("p (k f) -> p k f", k=NOFF)

        def syn_mults(FPREV, groups):
            for g in groups:
                if g[0] == "run":
                    _, k0, L, dy, dz0, _ = g
                    nc.vector.tensor_tensor(
                        w4(P, k0, L), w4(W, k0, L), grp_field(FPREV, L, dy, dz0),
                        ALU.mult,
                    )
                else:
                    _, ka, kb, dy, dz0, fs, _ = g
                    nc.vector.tensor_tensor(
                        pairk(P, ka, kb), pairk(W, ka, kb),
                        grp_field(FPREV, 2, dy, dz0, kstride=fs),
                        ALU.mult,
                    )

        def stdp_mults(FPREV, pvx, groups):
            for g in groups:
                if g[0] == "run":
                    _, k0, L, dy, dz0, pdx = g
                    nc.vector.tensor_tensor(
                        w4(P, k0, L), prev_bcast(pvx, FPREV, L, pdx),
                        grp_field(QF, L, dy, dz0), ALU.mult,
                    )
                else:
                    _, ka, kb, dy, dz0, fs, prev = g
                    if prev == "p01":
                        pv = pvx4(pvx)[:, 0:2]
                    elif prev == "p23":
                        pv = pvx4(pvx)[:, 2:4]
                    else:
                        pv = fint(FPREV).unsqueeze(1).to_broadcast(
                            [P128, 2, YH, NZ]
                        )
                    nc.vector.tensor_tensor(
                        pairk(P, ka, kb), pv,
                        grp_field(QF, 2, dy, dz0, kstride=fs),
                        ALU.mult,
                    )

        def w_update(ranges, hi_t):
            for (r0, r1), pair in ranges:
                if pair is not None:
                    wv = blockpair(W, r0, r1, pair)
                    pv = blockpair(P, r0, r1, pair)
                else:
                    wv, pv = wf(W, r0, r1), wf(P, r0, r1)
                nc.vector.tensor_tensor(wv, wv, pv, ALU.add)
                nc.vector.tensor_scalar(wv, wv, hi_t, 0.0, ALU.min, ALU.max)

        RANGES_HF = (((0, 3), 5), ((10, 14), None), ((22, 24), None))

        # ---- init ----
        nc.vector.memset(FA[:], 0.0)
        nc.vector.memset(FB[:], 0.0)
        nc.vector.memset(PVX2[0][:], 0.0)
        nc.vector.memset(PVX2[1][:], 0.0)
        nc.vector.memset(V[:], REST_V)
        nc.vector.memset(RST[:], RESET_V)
        nc.vector.memset(B30[:], -0.5 * MIDPOINT)
        nc.sync.dma_start(W[:], w0_d[:])
        nc.sync.dma_start(SM[:], sm_d[:])
        nc.sync.dma_start(MSKB[:], msk_d[:])
        XV = XINP.rearrange("p (t f) -> p t f", t=nsteps)
        xin_v = xin_d.rearrange("p (t f) -> p t f", t=nsteps)
        nc.sync.dma_start(XV[:, 0:1], xin_v[:, 0:1])
        nc.sync.dma_start(XV[:, 1:], xin_v[:, 1:])

        pid = nc.sync.partition_id()
        offL = nc.sync.snap((pid + NCORES - 1) % NCORES, min_val=0, max_val=NCORES - 1)
        offR = nc.sync.snap((pid + 1) % NCORES, min_val=0, max_val=NCORES - 1)

        fields = [FA, FB]
        for t in range(nsteps):
            FPREV = fields[t % 2]
            FOUT = fields[(t + 1) % 2]
            last = t == nsteps - 1
            S = SS[t % 2]
            pvx = PVX2[t % 2]       # this step's stdp reads pvx
            pvx_next = PVX2[(t + 1) % 2]

            # ---- syn: all 24 slot products were emitted during the
            # previous iteration; here just the PE k-reduction ----
            if t > 0:
                order = MM_ORDER_HF + MM_ORDER_H
                for i, k in enumerate(order):
                    nc.tensor.matmul(
                        TDP[:], smv[:, SM_SLICE[SLOTS[k][0]]], Pk[:, k],
                        start=(i == 0), stop=(i == len(order) - 1),
                    )
                s_t = float(np.float32(ONE_MINUS_DECAY * c[t - 1]))
                nc.vector.scalar_tensor_tensor(
                    SYN[:], TDP[:], s_t, XV[:, t], ALU.mult, ALU.add
                )
                nc.vector.scalar_tensor_tensor(
                    V[:], V[:], DECAY, SYN[:], ALU.mult, ALU.add
                )
            else:
                nc.vector.scalar_tensor_tensor(
                    V[:], V[:], DECAY, XV[:, t], ALU.mult, ALU.add
                )

            # ---- neuron update ----
            nc.vector.tensor_scalar(S[:], V[:], EXC_THR, None, ALU.is_ge)
            nc.sync.dma_start(spk_d[t], S[:])
            if last:
                continue
            nc.vector.tensor_scalar(SI[:], V[:], EXC_THR, None, ALU.is_ge)
            nc.vector.tensor_scalar(II[:], V[:], INH_THR, None, ALU.is_le)
            nc.scalar.activation(G[:], V[:], AT.Sigmoid, bias=B30[:, 0:1], scale=0.5)
            nc.vector.tensor_tensor(E[:], S[:], II[:], ALU.subtract)
            nc.vector.tensor_tensor(E[:], G[:], E[:], ALU.add)
            # out = clip01(...): boundary-strip rows first so the AllGather
            # input DMAs launch before the interior is written
            nc.vector.tensor_scalar(
                fint(FOUT)[0:NX, 0:2], c3(E)[0:NX, 0:2], 1.0, 0.0, ALU.min, ALU.max
            )
            nc.vector.tensor_scalar(
                fint(FOUT)[NX:P128, 2:4], c3(E)[NX:P128, 2:4],
                1.0, 0.0, ALU.min, ALU.max,
            )
            # ---- boundary strips -> AllGather (unmasked; edge-core wrap
            # garbage is neutralized by W'=0 on syn and the masked QF-halo
            # scale on STDP) ----
            agin = dr.tile([P128, 2 * NZ], DT, name=f"agin{t}")
            agout = dr.tile(
                [NCORES * P128, 2 * NZ], DT, addr_space="Shared", name=f"agout{t}"
            )
            agv = agin.rearrange("p (r z) -> p r z", z=NZ)
            nc.sync.dma_start(agv[0:NX], f3(FOUT)[0:NX, 2:4, 2 : 2 + NZ])
            nc.sync.dma_start(agv[NX:P128], f3(FOUT)[NX:P128, 4:6, 2 : 2 + NZ])
            nc.vector.tensor_scalar(
                fint(FOUT)[0:NX, 2:4], c3(E)[0:NX, 2:4], 1.0, 0.0, ALU.min, ALU.max
            )
            nc.vector.tensor_scalar(
                fint(FOUT)[NX:P128, 0:2], c3(E)[NX:P128, 0:2],
                1.0, 0.0, ALU.min, ALU.max,
            )
            nc.vector.copy_predicated(V[:], SI[:], RST[:])
            nc.gpsimd.collective_compute(
                "AllGather",
                ALU.bypass,
                replica_groups=[list(range(NCORES))],
                ins=[agin.opt()],
                outs=[agout.opt()],
            )
            nc.sync.dma_start(f3(FOUT)[0:NX, 6:8, 2 : 2 + NZ],
                              f3(FOUT)[NX:P128, 2:4, 2 : 2 + NZ])
            nc.sync.dma_start(f3(FOUT)[NX:P128, 0:2, 2 : 2 + NZ],
                              f3(FOUT)[0:NX, 4:6, 2 : 2 + NZ])

            # ---- overlap window: halo-free STDP + next-step prefetches ----
            do_stdp = t > 0
            if do_stdp:
                a_t = float(np.float32((ETA_LTP + ETA_LTD) / c[t]))
                b_t = float(np.float32(-ETA_LTD / c[t]))
                hi_t = float(np.float32(1.0 / c[t]))
                qf3 = f3(QF)
                fo3 = f3(FOUT)
                nc.vector.tensor_scalar(
                    qf3[:, 2:6], fo3[:, 2:6], a_t, b_t, ALU.mult, ALU.add
                )
                # per-step masked QF-halo scales (zero invalid halo sources)
                nc.vector.tensor_scalar(SA[:], MSKB[:], a_t, None, ALU.mult)
                stdp_mults(FPREV, pvx, GROUPS_HF)
                w_update(RANGES_HF, hi_t)

            # prefetch x-shifted prev chunks for next step's stdp
            if t + 1 < nsteps - 1:
                pvs = pvx_next.rearrange("p (s f) -> p s f", s=4)
                for dxv, sl in PVX_SLICE.items():
                    xshift_dma(nc.scalar, pvs[:, sl], fint(FOUT), -dxv)

            # next step's halo-free syn products (fills the collective wait)
            if t + 1 < nsteps:
                syn_mults(FOUT, GROUPS_HF)

            # ---- halo in from neighbors ----
            agf = agout.rearrange("p (r z) -> p r z", z=NZ)
            nc.sync.dma_start(
                f3(FOUT)[0:NX, 0:2, 2 : 2 + NZ],
                agf[bass.ds(offL * P128 + NX, NX)],
            )
            nc.sync.dma_start(
                f3(FOUT)[NX:P128, 6:8, 2 : 2 + NZ],
                agf[bass.ds(offR * P128, NX)],
            )

            # ---- halo-dependent STDP, interleaved per slot-range with the
            # NEXT step's halo syn products so only the last range's mult
            # sits between the final clip and the PE reduction ----
            if do_stdp:
                nc.vector.tensor_scalar(
                    qf3[:, 0:2], fo3[:, 0:2], SA[:, 0:1], b_t, ALU.mult, ALU.add
                )
                nc.vector.tensor_scalar(
                    qf3[:, 6:8], fo3[:, 6:8], SA[:, 1:2], b_t, ALU.mult, ALU.add
                )
            emit_next_syn = t + 1 < nsteps
            blocks = [
                ([("run", 14, 3, 1, 1, 0)], (((14, 17), None),)),
                ([("run", 17, 3, -1, 1, 0)], (((17, 20), None),)),
                ([("pair", 3, 8, 1, 0, 0, "p01"),
                  ("pair", 4, 9, -1, 0, 0, "p01")], (((3, 5), 8),)),
                ([("pair", 20, 21, 2, 0, 4 * 68, 0)], (((20, 22), None),)),
            ]
            for descs, ranges in blocks:
                if do_stdp:
                    stdp_mults(FPREV, pvx, descs)
                    w_update(ranges, hi_t)
                if emit_next_syn:
                    syn_mults(FOUT, descs)

    nc.compile()
    return nc


def _shard_inputs(external_input, edge_values, edge_rows, edge_cols, nsteps):
    ext = np.ascontiguousarray(np.asarray(external_input, dtype=np.float32))[:nsteps]
    vals = np.asarray(edge_values, dtype=np.float32)
    rows = np.asarray(edge_rows, dtype=np.int64)
    cols = np.asarray(edge_cols, dtype=np.int64)

    dlin = cols - rows
    offs_lin = np.array([d[0] * NY * NZ + d[1] * NZ + d[2] for d in SLOTS])
    k_of = {int(v): i for i, v in enumerate(offs_lin)}
    ke = np.array([k_of[int(v)] for v in dlin], dtype=np.int64)
    Wd = np.zeros((NOFF, N), dtype=np.float32)
    Wd[ke, cols] = vals
    Wd = Wd.reshape(NOFF, NX, NY, NZ)

    Wsrc = np.zeros_like(Wd)
    for k, (dx, _, _) in enumerate(SLOTS):
        if dx >= 0:
            Wsrc[k, : NX - dx] = Wd[k, dx:]
        else:
            Wsrc[k, -dx:] = Wd[k, : NX + dx]

    ext4 = ext.reshape(nsteps, NX, NY, NZ) * np.float32(ONE_MINUS_DECAY)

    # shift matrices: slices (+1,-1,+2,-2,identity); SM_s[p,m]=1 iff m=p+dx
    sm = np.zeros((2 * NX, 5, 2 * NX), dtype=np.float32)
    for s, dxv in enumerate((1, -1, 2, -2, 0)):
        for h in (0, 1):
            for xs in range(NX):
                xm = xs + dxv
                if 0 <= xm < NX:
                    sm[h * NX + xs, s, h * NX + xm] = 1.0
    sm = sm.reshape(2 * NX, 5 * 2 * NX).astype(NPDT)

    in_maps = []
    for cidx in range(NCORES):
        ylo = cidx * YS
        sub = Wsrc[:, :, ylo : ylo + YS, :]
        tr = sub.transpose(1, 0, 2, 3)
        wc = np.concatenate(
            [tr[:, :, :YH, :].reshape(NX, NOFF * CH),
             tr[:, :, YH:, :].reshape(NX, NOFF * CH)], axis=0
        ).astype(NPDT)
        esub = ext4[:, :, ylo : ylo + YS, :].transpose(1, 0, 2, 3)
        xc = np.concatenate(
            [esub[:, :, :YH, :].reshape(NX, nsteps * CH),
             esub[:, :, YH:, :].reshape(NX, nsteps * CH)], axis=0
        ).astype(np.float32)
        # halo-validity masks: col 0 gates field rows 0:2 (h0 <- left
        # neighbor, h1 <- intra); col 1 gates rows 6:8 (h0 <- intra,
        # h1 <- right neighbor)
        msk = np.ones((2 * NX, 2), dtype=np.float32)
        if cidx == 0:
            msk[:NX, 0] = 0.0
        if cidx == NCORES - 1:
            msk[NX:, 1] = 0.0
        in_maps.append(
            {"w0": np.ascontiguousarray(wc), "xin": np.ascontiguousarray(xc),
             "msk": msk, "sm": sm}
        )
    return in_maps


def kernel(external_input, edge_values, edge_rows, edge_cols, num_steps):
    nsteps = int(num_steps)
    if nsteps not in _CACHE:
        _CACHE[nsteps] = _build_graph(nsteps)
    nc = _CACHE[nsteps]

    in_maps = _shard_inputs(external_input, edge_values, edge_rows, edge_cols, nsteps)
    res = bass_utils.run_bass_kernel_spmd(
        nc,
        in_maps,
        core_ids=list(range(NCORES)),
        trace=bool(int(os.environ.get("BRAIN_TRACE", "0"))),
    )

    out = np.empty((nsteps, NX, NY, NZ), dtype=np.float32)
    for cidx in range(NCORES):
        ylo = cidx * YS
        spk = res.results[cidx]["spk"].reshape(nsteps, 2, NX, YH, NZ)
        out[:, :, ylo : ylo + YH, :] = spk[:, 0]
        out[:, :, ylo + YH : ylo + YS, :] = spk[:, 1]
    kernel.last_results = res
    return out.reshape(nsteps, N)
